# revision 8
# baseline (speedup 1.0000x reference)
"""EdgeEncoder Bass kernel for 8 Trainium2 NeuronCores.

Reference computation (per batch b, edge e):
    u = node_emb[b, idx_u[e]]; v = node_emb[b, idx_v[e]]
    x = [u, v, edge_sel[e]]                      # [257]
    h = relu(x @ W1 + b1)                        # [128]
    out = relu(h @ W2 + b2)                      # [128]

Sharding: data-parallel over batches; 2 cores per batch element share its
edges (host-balanced). Each core gathers its edge endpoints from HBM with
the custom dma_gather SWDGE instruction (full SDMA descriptor bandwidth).

dma_gather takes int16 indices, so the 100000-row node table is split into
4 windows of 25000 rows. Edges are bucketed on the host into 16 classes by
(u-window, v-window); each class is a stream of gather tiles whose u and v
indices are window-local. The host permutes edges into this layout and
inverts the permutation when assembling the output; class overflow beyond
the static per-class capacity (not expected at these sizes) is computed on
the host and patched in.

Per gather tile (T subtiles of 128 edges): two dma_gathers (u, v) pull
512B rows into [128, T*128] SBUF tiles; per group of 4 subtiles the PE
transposes u and v (fp32r), layer 1 runs as wide N=512 fp32r matmuls
(W1u, W1v, W1e stationary), ReLU+b1 on ACT (psum->sbuf, per-partition
bias in the transposed layout), layer 2 keeps W2 stationary producing the
transposed output (N=512), ReLU+b2 on ACT, PE transposes back (fp32),
DVE copies psum->sbuf, and one store DMA per tile writes rows with
contiguous multi-KB per-partition descriptors (edge el = p*T + j layout).
"""

import contextlib

import numpy as np

import concourse.bacc as bacc
import concourse.mybir as mybir
import concourse.tile as tile
from concourse.bass_utils import run_bass_kernel_spmd

B, N, D, E = 4, 100000, 128, 262144
HID, OUT = 128, 128
NCORES = 8
P = 128

E_CORE = (B * E) // NCORES  # 131072 edges per core (before rebalancing)
WIN = 25000                 # node-table window (int16-indexable)
NW = 4                      # windows
NCLS = NW * NW              # 16 classes
CAP_SUB = 66                # subtiles of 128 per class stream (8448 edges)
TILES = (16, 16, 16, 16, 2)  # subtiles per gather tile within a class
S_OFF = (0, 16, 32, 48, 64)
CAP = CAP_SUB * P           # 8448
SUB_DEV = NCLS * CAP_SUB    # 1056 subtiles per core
E_DEV = SUB_DEV * P         # 135168 device edge slots per core
GROUP = 4                   # subtiles per compute group

F32 = mybir.dt.float32
F32R = mybir.dt.float32r
I16 = mybir.dt.int16


def _build_program(reps=None):
    nc = bacc.Bacc(None, target_bir_lowering=False, debug=False)

    node = nc.dram_tensor("node", [N, D], F32R, kind="ExternalInput")
    ui = nc.dram_tensor("ui", [P, SUB_DEV * 8], I16, kind="ExternalInput")
    vi = nc.dram_tensor("vi", [P, SUB_DEV * 8], I16, kind="ExternalInput")
    esel = nc.dram_tensor("esel", [SUB_DEV, P], F32R, kind="ExternalInput")
    w1u = nc.dram_tensor("w1u", [D, HID], F32R, kind="ExternalInput")
    w1v = nc.dram_tensor("w1v", [D, HID], F32R, kind="ExternalInput")
    w1e = nc.dram_tensor("w1e", [1, HID], F32R, kind="ExternalInput")
    b1c = nc.dram_tensor("b1c", [HID, 1], F32, kind="ExternalInput")
    w2 = nc.dram_tensor("w2", [HID, OUT], F32R, kind="ExternalInput")
    b2c = nc.dram_tensor("b2c", [OUT, 1], F32, kind="ExternalInput")
    ident = nc.dram_tensor("ident", [P, P], F32, kind="ExternalInput")
    identr = nc.dram_tensor("identr", [P, P], F32R, kind="ExternalInput")
    out = nc.dram_tensor("out", [E_DEV, OUT], F32, kind="ExternalOutput")

    relu = mybir.ActivationFunctionType.Relu

    with tile.TileContext(nc) as tc:
        with (
            tc.tile_pool(name="const", bufs=1) as constp,
            tc.tile_pool(name="gather", bufs=3) as gatherp,
            tc.tile_pool(name="eselp", bufs=2) as eselp,
            tc.tile_pool(name="sb", bufs=3) as sbp,
            tc.tile_pool(name="fo", bufs=2) as fop,
            tc.tile_pool(name="ps", bufs=1, space="PSUM") as psp,
            tc.tile_pool(name="ps2", bufs=2, space="PSUM") as ps2p,
        ):
            ui_sb = constp.tile([P, SUB_DEV * 8], I16)
            nc.sync.dma_start(ui_sb[:], ui[:])
            vi_sb = constp.tile([P, SUB_DEV * 8], I16)
            nc.sync.dma_start(vi_sb[:], vi[:])
            w1u_sb = constp.tile([D, HID], F32R)
            nc.sync.dma_start(w1u_sb[:], w1u[:])
            w1v_sb = constp.tile([D, HID], F32R)
            nc.sync.dma_start(w1v_sb[:], w1v[:])
            w1e_sb = constp.tile([1, HID], F32R)
            nc.sync.dma_start(w1e_sb[:], w1e[:])
            b1_sb = constp.tile([HID, 1], F32)
            nc.sync.dma_start(b1_sb[:], b1c[:])
            w2_sb = constp.tile([HID, OUT], F32R)
            nc.sync.dma_start(w2_sb[:], w2[:])
            b2_sb = constp.tile([OUT, 1], F32)
            nc.sync.dma_start(b2_sb[:], b2c[:])
            id_sb = constp.tile([P, P], F32)
            nc.sync.dma_start(id_sb[:], ident[:])
            idr_sb = constp.tile([P, P], F32R)
            nc.sync.dma_start(idr_sb[:], identr[:])

            loop_cm = (
                tc.For_i(0, reps, 1, hint_engines=tuple(nc.engines))
                if reps is not None
                else contextlib.nullcontext()
            )
            with loop_cm:
                for c in range(NCLS):
                    wu, wv = c // NW, c % NW
                    src_u = node[wu * WIN : (wu + 1) * WIN, :]
                    src_v = node[wv * WIN : (wv + 1) * WIN, :]
                    for k, T in enumerate(TILES):
                        sb_base = c * CAP_SUB + S_OFF[k]
                        nidx = T * P
                        gtu = gatherp.tile([P, 16 * P], F32R, tag="gtu")
                        nc.gpsimd.dma_gather(
                            out_ap=gtu[:, : T * P].rearrange(
                                "p (j f) -> p j f", f=P
                            ),
                            in_ap=src_u,
                            idxs_ap=ui_sb[:, sb_base * 8 : (sb_base + T) * 8],
                            num_idxs=nidx,
                            num_idxs_reg=nidx,
                            elem_size=D,
                            single_packet=False,
                        )
                        gtv = gatherp.tile([P, 16 * P], F32R, tag="gtv")
                        nc.gpsimd.dma_gather(
                            out_ap=gtv[:, : T * P].rearrange(
                                "p (j f) -> p j f", f=P
                            ),
                            in_ap=src_v,
                            idxs_ap=vi_sb[:, sb_base * 8 : (sb_base + T) * 8],
                            num_idxs=nidx,
                            num_idxs_reg=nidx,
                            elem_size=D,
                            single_packet=False,
                        )
                        es = eselp.tile([1, 16 * P], F32R, tag="es")
                        nc.sync.dma_start(
                            es[:, : T * P], esel[sb_base : sb_base + T, :]
                        )
                        fo = fop.tile([P, 16 * P], F32, tag="fo")

                        for gi in range((T + GROUP - 1) // GROUP):
                            g = min(GROUP, T - gi * GROUP)
                            W = g * P
                            c0 = gi * GROUP * P  # column base within tile
                            tpu = psp.tile([P, GROUP * P], F32R, tag="tpu")
                            tpv = psp.tile([P, GROUP * P], F32R, tag="tpv")
                            for j in range(g):
                                cc = c0 + j * P
                                nc.tensor.transpose(
                                    tpu[:, j * P : (j + 1) * P],
                                    gtu[:, cc : cc + P],
                                    idr_sb[:],
                                )
                                nc.tensor.transpose(
                                    tpv[:, j * P : (j + 1) * P],
                                    gtv[:, cc : cc + P],
                                    idr_sb[:],
                                )
                            uT = sbp.tile([P, GROUP * P], F32R, tag="uT")
                            nc.vector.tensor_copy(uT[:, :W], tpu[:, :W])
                            vT = sbp.tile([P, GROUP * P], F32R, tag="vT")
                            nc.vector.tensor_copy(vT[:, :W], tpv[:, :W])

                            hp = psp.tile([HID, GROUP * P], F32, tag="hp")
                            nc.tensor.matmul(
                                hp[:, :W], lhsT=w1u_sb[:], rhs=uT[:, :W],
                                start=True, stop=False,
                            )
                            nc.tensor.matmul(
                                hp[:, :W], lhsT=w1v_sb[:], rhs=vT[:, :W],
                                start=False, stop=False,
                            )
                            nc.tensor.matmul(
                                hp[:, :W], lhsT=w1e_sb[:],
                                rhs=es[0:1, c0 : c0 + W],
                                start=False, stop=True,
                            )
                            hT = sbp.tile([HID, GROUP * P], F32R, tag="hT")
                            nc.scalar.activation(
                                hT[:, :W], hp[:, :W], relu, bias=b1_sb[:]
                            )

                            op_ = ps2p.tile([OUT, GROUP * P], F32, tag="op")
                            nc.tensor.matmul(
                                op_[:, :W], lhsT=w2_sb[:], rhs=hT[:, :W],
                                start=True, stop=True,
                            )
                            oT = sbp.tile([OUT, GROUP * P], F32, tag="oT")
                            nc.scalar.activation(
                                oT[:, :W], op_[:, :W], relu, bias=b2_sb[:]
                            )

                            fp = ps2p.tile([P, GROUP * P], F32, tag="fp")
                            for j in range(g):
                                nc.tensor.transpose(
                                    fp[:, j * P : (j + 1) * P],
                                    oT[:, j * P : (j + 1) * P],
                                    id_sb[:],
                                )
                            nc.vector.tensor_copy(
                                fo[:, c0 : c0 + W], fp[:, :W]
                            )

                        r0 = sb_base * P
                        nc.sync.dma_start(
                            out[r0 : r0 + T * P, :].rearrange(
                                "(p j) f -> p (j f)", p=P, j=T
                            ),
                            fo[:, : T * P],
                        )

    nc.compile()
    return nc


_program_cache = {}


def _get_program(reps=None):
    if reps not in _program_cache:
        _program_cache[reps] = _build_program(reps)
    return _program_cache[reps]


def _layout_core(u, v, es):
    """Given this core's edge endpoint arrays (int32, clipped) and edge_sel
    values, build device inputs and the stream->edge mapping.

    Returns (ui, vi, esel_dev, strm, overflow_positions)."""
    ne = u.shape[0]
    wu = u // WIN
    wv = v // WIN
    cls = wu * NW + wv

    order = np.argsort(cls, kind="stable")
    cls_sorted = cls[order]
    counts = np.bincount(cls_sorted, minlength=NCLS)
    starts = np.concatenate([[0], np.cumsum(counts)])

    strm = np.full((NCLS, CAP), -1, dtype=np.int64)
    overflow = []
    for c in range(NCLS):
        ids = order[starts[c] : starts[c + 1]]
        if len(ids) > CAP:
            overflow.append(ids[CAP:])
            ids = ids[:CAP]
        strm[c, : len(ids)] = ids
    overflow = np.concatenate(overflow) if overflow else np.empty(0, np.int64)

    u_loc = (u - wu * WIN).astype(np.int16)
    v_loc = (v - wv * WIN).astype(np.int16)

    ui_dev = np.zeros((P, SUB_DEV * 8), dtype=np.int16)
    vi_dev = np.zeros((P, SUB_DEV * 8), dtype=np.int16)
    es_dev = np.zeros((SUB_DEV, P), dtype=np.float32)

    for c in range(NCLS):
        for k, T in enumerate(TILES):
            sb = c * CAP_SUB + S_OFF[k]
            seg = strm[c, S_OFF[k] * P : (S_OFF[k] + T) * P]
            seg_pj = seg.reshape(P, T)           # stream pos s = p*T + j
            lst = seg_pj.T.reshape(-1)           # gather list pos n = j*128 + p
            valid = lst >= 0
            ul = np.where(valid, u_loc[np.where(valid, lst, 0)], 0)
            vl = np.where(valid, v_loc[np.where(valid, lst, 0)], 0)
            # wrapped [16, n/16] replicated to 128 partitions
            uw = np.tile(ul.reshape(T * 8, 16).T, (8, 1))
            vw = np.tile(vl.reshape(T * 8, 16).T, (8, 1))
            ui_dev[:, sb * 8 : (sb + T) * 8] = uw
            vi_dev[:, sb * 8 : (sb + T) * 8] = vw
            vpj = seg_pj >= 0
            es_blk = np.where(vpj, es[np.where(vpj, seg_pj, 0)], 0.0)
            es_dev[sb : sb + T, :] = es_blk.T    # row j, col p

    return ui_dev, vi_dev, es_dev, strm.reshape(-1), overflow


def _make_in_maps(node_emb, edge_index, edge_sel, W1, b1, W2, b2):
    node_emb = np.asarray(node_emb, dtype=np.float32)
    edge_index = np.asarray(edge_index)
    edge_sel = np.asarray(edge_sel, dtype=np.float32)
    W1 = np.asarray(W1, dtype=np.float32)
    b1 = np.asarray(b1, dtype=np.float32)
    W2 = np.asarray(W2, dtype=np.float32)
    b2 = np.asarray(b2, dtype=np.float32)

    w1u = np.ascontiguousarray(W1[0:D])
    w1v = np.ascontiguousarray(W1[D : 2 * D])
    w1e = np.ascontiguousarray(W1[2 * D : 2 * D + 1])
    b1c = np.ascontiguousarray(b1.reshape(HID, 1))
    b2c = np.ascontiguousarray(b2.reshape(OUT, 1))
    ident = np.eye(P, dtype=np.float32)

    in_maps = []
    meta = []  # (batch, edge_ids_in_batch, strm, overflow_ids)
    for bb in range(B):
        ei = np.clip(edge_index[bb], 0, N - 1).astype(np.int32)
        esb = edge_sel[bb, :, 0].astype(np.float32)
        # balance the two cores of this batch per class
        cls = (ei[:, 0] // WIN) * NW + ei[:, 1] // WIN
        order = np.argsort(cls, kind="stable")
        half = np.zeros(E, dtype=bool)
        # alternate within each class for an even split
        pos_in_class = np.empty(E, np.int64)
        cls_sorted = cls[order]
        counts = np.bincount(cls_sorted, minlength=NCLS)
        starts = np.concatenate([[0], np.cumsum(counts)])
        for c in range(NCLS):
            ids = order[starts[c] : starts[c + 1]]
            half[ids[1::2]] = True
        for hh in range(2):
            ids = np.where(half == bool(hh))[0]
            uu, vv, ee = ei[ids, 0], ei[ids, 1], esb[ids]
            ui_dev, vi_dev, es_dev, strm, ovf = _layout_core(uu, vv, ee)
            in_maps.append(
                {
                    "node": np.ascontiguousarray(node_emb[bb]),
                    "ui": ui_dev,
                    "vi": vi_dev,
                    "esel": es_dev,
                    "w1u": w1u,
                    "w1v": w1v,
                    "w1e": w1e,
                    "b1c": b1c,
                    "w2": W2,
                    "b2c": b2c,
                    "ident": ident,
                    "identr": ident,
                }
            )
            meta.append((bb, ids, strm, ids[ovf] if len(ovf) else ovf))
    return in_maps, meta


def _assemble_out(results, meta, inputs):
    node_emb, edge_index, edge_sel, W1, b1, W2, b2 = inputs
    out = np.empty((B, E, OUT), dtype=np.float32)
    for c_i, (bb, ids, strm, ovf_ids) in enumerate(meta):
        dev = results[c_i]["out"]  # [E_DEV, OUT]
        mask = strm >= 0
        out[bb, ids[strm[mask]]] = dev[mask]
        if len(ovf_ids):
            ei = np.clip(edge_index[bb, ovf_ids], 0, N - 1)
            uu = node_emb[bb][ei[:, 0]]
            vv = node_emb[bb][ei[:, 1]]
            x = np.concatenate(
                [uu, vv, edge_sel[bb, ovf_ids]], axis=-1
            ).astype(np.float32)
            h = np.maximum(x @ W1 + b1, 0.0)
            out[bb, ovf_ids] = np.maximum(h @ W2 + b2, 0.0)
    return out


def kernel(node_emb, edge_index, edge_sel, W1, b1, W2, b2):
    node_emb = np.asarray(node_emb, dtype=np.float32)
    edge_index = np.asarray(edge_index)
    edge_sel = np.asarray(edge_sel, dtype=np.float32)
    in_maps, meta = _make_in_maps(
        node_emb, edge_index, edge_sel, W1, b1, W2, b2
    )
    nc = _get_program()
    res = run_bass_kernel_spmd(nc, in_maps, core_ids=list(range(NCORES)))
    return _assemble_out(
        res.results, meta, (node_emb, edge_index, edge_sel, W1, b1, W2, b2)
    )
